# revision 12
# baseline (speedup 1.0000x reference)
"""CrossAttention kernel for 8 trn2 NeuronCores.

Reference:
  q = x @ Wq          [n, vq, h]
  k = y @ Wk          [n, vk, h]
  v = y @ Wv          [n, vk, c]
  out = softmax(q k^T / sqrt(h)) @ v        [n, vq, c]
with N=4, VQ=VK=4096, C=128, H=64, fp32.

Sharding: 8 cores = 4 batches x 2 query halves. Each core gets
x-shard [2048, 128] and the full y[n] [4096, 128], computes out-shard
[2048, 128].

Per-core dataflow (all matmuls fp32r = full-rate PE):
  - x/y loaded CONTIGUOUSLY: partition p holds rows 4p..4p+3 of each
    512-row chunk, so each chunk is one dense 256KB DMA. The resulting
    within-chunk column permutation (xT col t*128+p <-> row 4p+t) is
    self-inverse at the output DMA, and irrelevant for y (all vk uses
    are permutation-invariant reductions).
  - transpose x,y tiles on PE (exact) -> xT [c, vq], yT [c, vk]
  - qT [h, vq] = Wq^T xT ; kT [h, vk] = Wk^T yT (rows duplicated for
    dual-quadrant score matmuls); v [vk, c] = yT_block^T [Wv|Wv]
    (duplicated moving operand keeps f32r at full rate).
  - flash loop over vq tiles of 512:
      scoresT [vk_t=128, vq=512] = kT_tile^T qT_tile   (K=h=64, 2 vk
        tiles concurrently in the two PE row-halves)
      attnT = exp(scale * scoresT)  (ScalarE, PSUM->SBUF, f32r out)
      outT [c, 512] += v_tile^T attnT  over 32 vk tiles
      sums [1, 512] += ones^T attnT   (dual row-half pairs)
    per-j epilogue folded into the next items: sums -> row-transpose ->
    reciprocal; outT -> transpose -> scale -> DMA out, all overlapped
    with the next j's flash compute.
"""

import sys

sys.path.insert(0, "/opt/trn_rl_repo")

from contextlib import ExitStack

import numpy as np

import concourse.bass as bass
import concourse.tile as tile
from concourse import mybir
from concourse.bass_utils import run_bass_kernel_spmd
from concourse.masks import make_identity

F32 = mybir.dt.float32
F32R = mybir.dt.float32r
P = 128

N, VQ, VK, C, H = 4, 4096, 4096, 128, 64
VQ_PER = VQ // 2          # 2048 queries per core
SCALE = float(H) ** -0.5

# main-loop tiling
VQ_T = 512                # vq tile (psum free dim)
N_VQ_T = VQ_PER // VQ_T   # 4
N_VK_T = VK // P          # 32 vk tiles of 128
CHUNK = 2                 # vk tiles per exp chunk (row-packed pair)
NXCH = VQ_PER // 512      # 4 x chunks
NYCH = VK // 512          # 8 y chunks


def _split_multi_waits(nc):
    """walrus in this env supports one sync-wait per instruction; hoist
    extras onto same-engine NoOps inserted just before."""
    for fn in nc.m.functions:
        for bb in fn.blocks:
            out = []
            for inst in bb.instructions:
                si = inst.sync_info
                waits = list(si.on_wait) if si and si.on_wait else []
                if len(waits) > 1:
                    for w in waits[:-1]:
                        out.append(mybir.InstNoOp(
                            name=nc.get_next_instruction_name(),
                            engine=inst.engine,
                            ins=[], outs=[],
                            sync_info=mybir.SyncInfo(on_wait=[w], on_update=[]),
                        ))
                    inst.sync_info = mybir.SyncInfo(
                        on_wait=[waits[-1]],
                        on_update=list(si.on_update) if si.on_update else [],
                    )
                out.append(inst)
            bb.instructions = out


def _build():
    nc = bass.Bass()
    x_d = nc.declare_dram_parameter("x", [VQ_PER, C], F32, isOutput=False)
    y_d = nc.declare_dram_parameter("y", [VK, C], F32, isOutput=False)
    wq_d = nc.declare_dram_parameter("Wq", [C, H], F32, isOutput=False)
    wk_d = nc.declare_dram_parameter("Wk", [C, H], F32, isOutput=False)
    wv_d = nc.declare_dram_parameter("Wv", [C, C], F32, isOutput=False)
    o_d = nc.declare_dram_parameter("o", [VQ_PER, C], F32, isOutput=True)

    with tile.TileContext(nc) as tc, ExitStack() as ctx:
        const = ctx.enter_context(tc.tile_pool(name="const", bufs=1))
        persist = ctx.enter_context(tc.tile_pool(name="persist", bufs=1))

        # ---- constants ----
        ident = const.tile([P, P], F32)
        make_identity(nc, ident[:])
        ones_f = const.tile([P, 1], F32)
        nc.vector.memset(ones_f[:], 1.0)
        ones_r = const.tile([P, 1], F32R)
        nc.vector.tensor_copy(ones_r[:], ones_f[:])
        # preload the exp table set off the critical path
        dummy = const.tile([1, 1], F32)
        nc.scalar.activation(
            dummy[0:1, :], ones_f[0:1, :],
            mybir.ActivationFunctionType.Exp, scale=1.0)

        # ---- persistent tensors ----
        # one tile per DMA chunk: same-tile writes serialize on the DGE,
        # separate tiles let all input transfers pipeline.
        raw_x = [persist.tile([P, 4, P], F32, name=f"raw_x{i}")
                 for i in range(NXCH)]
        raw_y = [persist.tile([P, 4, P], F32, name=f"raw_y{i}")
                 for i in range(NYCH)]
        qT = persist.tile([P, VQ_PER], F32R)          # [128, 2048] rows 64:128 dup
        kT = persist.tile([P, VK], F32R)              # [128, 4096] rows 64:128 dup
        v_sb = persist.tile([P, N_VK_T * C], F32R)    # [128, 32*128] vk-major
        attnT = persist.tile([P, N_VK_T * VQ_T], F32R)  # [128, 32*512] per vq tile
        outT = persist.tile([P, VQ_T], F32)           # [c, 512] per-j
        out_sb = persist.tile([P, VQ_T], F32)         # [128, 4*128] per-j
        srow = persist.tile([P, VQ_T], F32)           # sum rows: p0=A, p64=B
        sums_sb = persist.tile([P, 2 * N_VQ_T], F32)  # transposed sums [128, 4+4]
        rsum = persist.tile([P, N_VQ_T], F32)

        # ---- input DMAs: contiguous 256KB chunks, x0 + weights first ----
        nc.sync.dma_start(
            raw_x[0][:],
            x_d[0:512, :].rearrange("(p t) c -> p t c", p=P))
        wq_s = const.tile([P, H], F32)
        wk_s = const.tile([P, H], F32)
        wv_s = const.tile([P, C], F32)
        nc.sync.dma_start(wq_s[:], wq_d[:])
        nc.sync.dma_start(wk_s[:], wk_d[:])
        nc.sync.dma_start(wv_s[:], wv_d[:])
        for ch in range(NYCH):
            nc.sync.dma_start(
                raw_y[ch][:],
                y_d[ch * 512:(ch + 1) * 512, :]
                .rearrange("(p t) c -> p t c", p=P))
        for ch in range(1, NXCH):
            nc.sync.dma_start(
                raw_x[ch][:],
                x_d[ch * 512:(ch + 1) * 512, :]
                .rearrange("(p t) c -> p t c", p=P))

        w_r = const.tile([P, 4 * H + 2 * C], F32R)
        nc.vector.tensor_copy(w_r[:, 0:H], wq_s[:])
        nc.vector.tensor_copy(w_r[:, H:2 * H], wq_s[:])
        nc.vector.tensor_copy(w_r[:, 2 * H:3 * H], wk_s[:])
        nc.vector.tensor_copy(w_r[:, 3 * H:4 * H], wk_s[:])
        nc.vector.tensor_copy(w_r[:, 4 * H:4 * H + C], wv_s[:])
        nc.vector.tensor_copy(w_r[:, 4 * H + C:], wv_s[:])
        wqq_r = w_r[:, 0:2 * H]          # [Wq | Wq] -> duplicated qT rows
        wkk_r = w_r[:, 2 * H:4 * H]      # [Wk | Wk] -> duplicated kT rows
        wvv_r = w_r[:, 4 * H:]           # [Wv | Wv] -> 256-wide moving

        # ---- phase 1: transpose + project ----
        with ExitStack() as pctx:
            tp_ps = pctx.enter_context(
                tc.tile_pool(name="tp_ps", bufs=2, space="PSUM"))
            pj_ps = pctx.enter_context(
                tc.tile_pool(name="pj_ps", bufs=2, space="PSUM"))
            v_ps = pctx.enter_context(
                tc.tile_pool(name="v_ps", bufs=2, space="PSUM"))
            xyT = pctx.enter_context(tc.tile_pool(name="xyT", bufs=3))

            def do_chunk(raw, ch, is_x):
                t_ps = tp_ps.tile([P, 512], F32, tag="tp")
                for b in range(4):
                    nc.tensor.transpose(
                        t_ps[:, b * P:(b + 1) * P], raw[:, b, :],
                        ident[:])
                t_sb = xyT.tile([P, 512], F32R, tag="t_sb")
                nc.scalar.copy(t_sb[:], t_ps[:])
                pj = pj_ps.tile([P, 512], F32, tag="pj")
                if is_x:
                    nc.tensor.matmul(
                        pj[:], wqq_r[:], t_sb[:], start=True, stop=True)
                    nc.vector.tensor_copy(
                        qT[:, ch * 512:(ch + 1) * 512], pj[:])
                else:
                    nc.tensor.matmul(
                        pj[:], wkk_r[:], t_sb[:], start=True, stop=True)
                    nc.vector.tensor_copy(
                        kT[:, ch * 512:(ch + 1) * 512], pj[:])
                    vp = v_ps.tile([P, 1024], F32, tag="vp")
                    for b in range(4):
                        nc.tensor.matmul(
                            vp[:, b * 256:(b + 1) * 256],
                            t_sb[:, b * P:(b + 1) * P], wvv_r[:],
                            start=True, stop=True)
                    nc.vector.tensor_copy(
                        v_sb[:, ch * 512:(ch + 1) * 512]
                        .rearrange("p (t c) -> p t c", c=C),
                        vp[:].rearrange("p (t c) -> p t c", c=2 * C)
                        [:, :, 0:C])

            do_chunk(raw_x[0], 0, True)
            for ch in range(NYCH):
                do_chunk(raw_y[ch], ch, False)
            for ch in range(1, NXCH):
                do_chunk(raw_x[ch], ch, True)

        # ---- phase 2: flash loop over vq tiles, epilogue folded in ----
        with ExitStack() as mctx:
            sc_ps = mctx.enter_context(
                tc.tile_pool(name="sc_ps", bufs=2, space="PSUM"))
            pv_ps = mctx.enter_context(
                tc.tile_pool(name="pv_ps", bufs=1, space="PSUM"))
            sm_ps = mctx.enter_context(
                tc.tile_pool(name="sm_ps", bufs=1, space="PSUM"))
            sm2_ps = mctx.enter_context(
                tc.tile_pool(name="sm2_ps", bufs=1, space="PSUM"))
            epi_ps = mctx.enter_context(
                tc.tile_pool(name="epi_ps", bufs=1, space="PSUM"))

            pv_tiles = [None] * N_VQ_T
            epi_tiles = [None] * N_VQ_T
            starts = list(range(0, N_VK_T, CHUNK))
            work = [(j, s) for j in range(N_VQ_T) for s in starts]

            def emit_scores_exp(j, s):
                sc = sc_ps.tile([P, CHUNK * VQ_T], F32, tag="sc")
                nc.tensor.matmul(
                    sc[:, 0:VQ_T],
                    kT[0:64, s * P:(s + 1) * P],
                    qT[0:64, j * VQ_T:(j + 1) * VQ_T],
                    start=True, stop=True)
                nc.tensor.matmul(
                    sc[:, VQ_T:2 * VQ_T],
                    kT[64:128, (s + 1) * P:(s + 2) * P],
                    qT[64:128, j * VQ_T:(j + 1) * VQ_T],
                    start=True, stop=True, tile_position=(64, 0))
                nc.scalar.activation(
                    attnT[:, s * VQ_T:(s + 2) * VQ_T],
                    sc[:],
                    mybir.ActivationFunctionType.Exp, scale=SCALE)

            def emit_pv_sm(j, s):
                if s == 0:
                    pv = pv_ps.tile([P, VQ_T], F32, tag="pv", name=f"pv{j}")
                    sm = sm_ps.tile([P, VQ_T], F32, tag="sm", name=f"sm{j}")
                    sm2 = sm2_ps.tile([P, VQ_T], F32, tag="sm2", name=f"sm2_{j}")
                    pv_tiles[j] = (pv, sm, sm2)
                pv, sm, sm2 = pv_tiles[j]
                for ii in range(CHUNK):
                    i = s + ii
                    a_sl = attnT[:, i * VQ_T:(i + 1) * VQ_T]
                    nc.tensor.matmul(
                        pv[:], v_sb[:, i * C:(i + 1) * C], a_sl,
                        start=(i == 0), stop=(i == N_VK_T - 1))
                for ii in range(CHUNK):
                    i = s + ii
                    a_sl = attnT[:, i * VQ_T:(i + 1) * VQ_T]
                    # concurrent pair: lower half rows -> sm, upper -> sm2
                    nc.tensor.matmul(
                        sm[0:1, :], ones_r[0:64, :], a_sl[0:64, :],
                        start=(i == 0), stop=(i == N_VK_T - 1))
                    nc.tensor.matmul(
                        sm2[0:1, :], ones_r[64:128, :], a_sl[64:128, :],
                        start=(i == 0), stop=(i == N_VK_T - 1),
                        tile_position=(64, 0))

            def epi_part1(j):
                pv, sm, sm2 = pv_tiles[j]
                epi = epi_ps.tile([P, VQ_T], F32, tag="epi", name=f"epi{j}")
                epi_tiles[j] = epi
                nc.vector.tensor_copy(srow[0:1, :], sm[0:1, :])
                nc.vector.tensor_tensor(
                    out=srow[0:1, :], in0=srow[0:1, :], in1=sm2[0:1, :],
                    op=mybir.AluOpType.add)
                nc.vector.tensor_copy(outT[:], pv[:])
                for t in range(4):
                    nc.tensor.transpose(
                        epi[:, t:t + 1], srow[0:1, t * P:(t + 1) * P],
                        ones_f[0:1, 0:1])
                nc.vector.tensor_copy(sums_sb[:, 0:4], epi[:, 0:4])
                nc.vector.reciprocal(rsum[:], sums_sb[:, 0:4])

            def epi_part2(j):
                epi = epi_tiles[j]
                for t in range(4):
                    nc.tensor.transpose(
                        epi[:, t * P:(t + 1) * P],
                        outT[:, t * P:(t + 1) * P], ident[:])
                for t in range(4):
                    nc.vector.tensor_scalar(
                        out=out_sb[:, t * P:(t + 1) * P],
                        in0=epi[:, t * P:(t + 1) * P],
                        scalar1=rsum[:, t:t + 1], scalar2=None,
                        op0=mybir.AluOpType.mult)
                nc.sync.dma_start(
                    o_d[j * VQ_T:(j + 1) * VQ_T, :]
                    .rearrange("(p t) c -> p t c", p=P),
                    out_sb[:].rearrange("p (t c) -> p t c", c=C))

            pend2 = None
            for n, (j, s) in enumerate(work):
                emit_scores_exp(j, s)
                if n > 0:
                    pj, ps = work[n - 1]
                    emit_pv_sm(pj, ps)
                    if pend2 is not None:
                        epi_part2(pend2)
                        pend2 = None
                    if ps == starts[-1]:
                        epi_part1(pj)
                        pend2 = pj
            emit_pv_sm(*work[-1])
            epi_part1(work[-1][0])
            epi_part2(work[-1][0])

    _split_multi_waits(nc)
    return nc


_NC = None


def _get_nc():
    global _NC
    if _NC is None:
        _NC = _build()
    return _NC


def kernel(x, y, Wq, Wk, Wv):
    x = np.ascontiguousarray(x, dtype=np.float32)
    y = np.ascontiguousarray(y, dtype=np.float32)
    Wq = np.ascontiguousarray(Wq, dtype=np.float32)
    Wk = np.ascontiguousarray(Wk, dtype=np.float32)
    Wv = np.ascontiguousarray(Wv, dtype=np.float32)

    nc = _get_nc()
    core_ids = list(range(8))
    in_maps = []
    for core in core_ids:
        n, half = core // 2, core % 2
        in_maps.append({
            "x": x[n, half * VQ_PER:(half + 1) * VQ_PER, :],
            "y": y[n],
            "Wq": Wq, "Wk": Wk, "Wv": Wv,
        })
    res = run_bass_kernel_spmd(nc, in_maps, core_ids)
    out = np.empty((N, VQ, C), dtype=np.float32)
    for core in core_ids:
        n, half = core // 2, core % 2
        out[n, half * VQ_PER:(half + 1) * VQ_PER, :] = res.results[core]["o"]
    return out
